# revision 26
# baseline (speedup 1.0000x reference)
"""minGRU cell kernel for 8 Trainium2 NeuronCores.

Math (per batch b, all in linear domain — the recurrence is a convex
combination of positive values, so no log-space is needed):
    gh[s, :] = x[s, :] @ W.T + b          # (S, 2H)
    gate, hidden = gh[:, :H], gh[:, H:]
    z = sigmoid(gate);  a = 1 - z = sigmoid(-gate)
    g(hidden) = relu(hidden) + min(sigmoid(hidden), 0.5)
    h_t = a_t * h_{t-1} + z_t * g_t       # scan over s

Distribution: pure data parallel over B (8 batches -> 8 cores).

Device layout: channels on SBUF partitions, time on the free dim:
    out[o, s] = sum_i WT[i, o] * xT[i, s]
so the matmul result lands directly in the layout the DVE
tensor_tensor_scan instruction needs.  Matmuls run as float32r
(full-rate fp32 PE mode).

Perf structure (vs the naive version):
  * W is packed host-side per output-tile (o-major), so the first
    j-chain only needs 1 MB of W + 2.1 MB of x before the PE can
    start — the old k-major layout needed all 8.4 MB of W first
    (26 us of PE idle at startup).
  * DMA descriptor issue is spread across three queues (W on the
    Vector queue, x on Sync, consts + output stores on GpSimd) so
    descriptor serialization never gates the startup transfers.
  * Within each (chunk, j) iteration the hidden chain runs BEFORE the
    gate chain: the post-matmul serial tail (sigmoid/relu/min-add on
    the hidden projection) overlaps the gate matmuls, and the final
    chunk's post-ops are split into 256-column halves, shortening the
    end-of-kernel drain.
"""

from contextlib import ExitStack

import numpy as np

import concourse.bass as bass
import concourse.bacc as bacc
import concourse.mybir as mybir
import concourse.tile as tile
from concourse.bass_utils import run_bass_kernel_spmd

B, S, DIN, DH = 8, 4096, 1024, 1024
CH = 512                 # time-chunk (free dim of each matmul / scan)
NCHUNK = S // CH         # 8
KT = DIN // 128          # 8 contraction tiles
JT = DH // 128           # 8 channel tiles (per gate/hidden half)

F32 = mybir.dt.float32
F32R = mybir.dt.float32r
AF = mybir.ActivationFunctionType
OP = mybir.AluOpType

_prog_cache = {}


def _build_program() -> bass.Bass:
    nc = bacc.Bacc("TRN2", target_bir_lowering=False, debug=False,
                   num_devices=B)
    xt = nc.dram_tensor("xt", (KT, 128, S), F32R, kind="ExternalInput")
    # per-o packed weights: wp[o*128+p, k*128+c] = W[o*128+c, k*128+p]
    wp = nc.dram_tensor("wp", (2 * DH, DIN), F32R, kind="ExternalInput")
    bias = nc.dram_tensor("bias", (128, 2 * JT), F32, kind="ExternalInput")
    nbias = nc.dram_tensor("nbias", (128, 2 * JT), F32, kind="ExternalInput")
    h0 = nc.dram_tensor("h0", (128, JT), F32, kind="ExternalInput")
    out = nc.dram_tensor("out", (DH, S), F32, kind="ExternalOutput")

    with ExitStack() as ctx:
        tc = ctx.enter_context(tile.TileContext(nc))
        cpool = ctx.enter_context(tc.tile_pool(name="const", bufs=1))
        wpool = ctx.enter_context(tc.tile_pool(name="w", bufs=1))
        xpool = ctx.enter_context(tc.tile_pool(name="x", bufs=2))
        spool = ctx.enter_context(tc.tile_pool(name="tmp", bufs=2))
        abpool = ctx.enter_context(tc.tile_pool(name="ab", bufs=3))
        hpool = ctx.enter_context(tc.tile_pool(name="h", bufs=2))
        # 2 tiles per chain, bufs=4 => exactly 2 chains in flight.  More
        # (bufs=8) lets the PE run 4 chains ahead of ACT, and the
        # then-permanent concurrent ACT psum reads slow every matmul's
        # accumulation by ~20% (measured 227 -> 272 ns cadence).
        ppool = ctx.enter_context(tc.tile_pool(name="psum", bufs=4, space="PSUM"))

        # Consts go over the (otherwise idle) GpSimd queue, then are
        # re-materialized on the engines that consume them (ACT for
        # bias/nbias, DVE for h0) so hot-loop instructions never carry
        # a DMA sync-wait.
        bias_d = cpool.tile([128, 2 * JT], F32, tag="bias_d")
        nc.gpsimd.dma_start(bias_d[:], bias[:, :])
        nbias_d = cpool.tile([128, 2 * JT], F32, tag="nbias_d")
        nc.gpsimd.dma_start(nbias_d[:], nbias[:, :])
        h0_d = cpool.tile([128, JT], F32, tag="h0_d")
        nc.gpsimd.dma_start(h0_d[:], h0[:, :])

        # Startup is DMA-bandwidth-bound: ~12.6 MB (W + x chunks 0/1)
        # must land in the first ~40 us.  Startup-critical input
        # transfers are sequenced in exact need order so later
        # transfers never steal bandwidth from earlier ones:
        #   w(j=0), x chunk 0, w(j=1..5), x chunk 1, w(j=6,7)
        # The first batch is split across the Sync AND Scalar queues
        # (two descriptors in flight ramps the DMA engines up faster);
        # everything after runs on Sync alone so the Scalar queue is
        # free for the ACT hot loop by the time psum drains start.
        def w_load(o, queue=nc.sync, per_k=False):
            w_t = wpool.tile([128, DIN], F32R, tag=f"w{o}")
            if per_k:
                # per-k-slice descriptors: the first matmul only waits
                # for the first 64 KB, not the whole 512 KB tile.
                for k in range(KT):
                    queue.dma_start(w_t[:, k * 128:(k + 1) * 128],
                                    wp[o * 128:(o + 1) * 128,
                                       k * 128:(k + 1) * 128])
            else:
                queue.dma_start(w_t[:], wp[o * 128:(o + 1) * 128, :])
            wts[o] = w_t

        def x_load(c, queue=nc.sync):
            # per-k descriptors: chunk-0 chains consume tiles
            # k-progressively as they land.
            s0 = c * CH
            xts = []
            for k in range(KT):
                x_t = xpool.tile([128, CH], F32R, tag=f"x{k}")
                queue.dma_start(x_t[:], xt[k, :, s0:s0 + CH])
                xts.append(x_t)
            xts_by_chunk[c] = [
                (lambda t: (lambda lo, hi: t[:, lo:hi]))(x_t) for x_t in xts]

        def x_load_big(c):
            # steady-state chunks: ONE descriptor for the whole chunk
            # (8 strided k-blocks) => one DMA semaphore instead of 8 on
            # the Tensor queue.
            s0 = c * CH
            xbig = xpool.tile([128, KT * CH], F32R, tag="xbig")
            nc.sync.dma_start(xbig[:].rearrange("p (k t) -> p k t", k=KT),
                              xt[:, :, s0:s0 + CH].rearrange("k p t -> p k t"))
            xts_by_chunk[c] = [
                (lambda kk: (lambda lo, hi:
                             xbig[:, kk * CH + lo:kk * CH + hi]))(k)
                for k in range(KT)]

        wts = [None] * (2 * JT)
        xts_by_chunk = {}
        # First batch split across queues: w(j=0) hidden tile per-k on
        # Sync while x chunk 0 issues per-k on Scalar, so the first
        # chain's k-step data (64 KB + 256 KB) lands pairwise and the
        # first matmul can start at ~9 us.
        w_load(JT + 0, per_k=True)
        x_load(0, nc.scalar)
        w_load(0)
        for j in range(1, 6):
            w_load(JT + j), w_load(j)
        x_load(1)
        for j in range(6, 8):
            w_load(JT + j), w_load(j)

        # Const copies: first ACT op needs bias_t at ~17 us.
        bias_t = cpool.tile([128, 2 * JT], F32, tag="bias")
        nc.scalar.copy(bias_t[:], bias_d[:])
        nbias_t = cpool.tile([128, 2 * JT], F32, tag="nbias")
        nc.scalar.copy(nbias_t[:], nbias_d[:])
        h0_t = cpool.tile([128, JT], F32, tag="h0")
        nc.vector.tensor_copy(h0_t[:], h0_d[:])

        prev_h = [None] * JT

        def chain(cs, j, subs, split_gate=False):
            """One j-chain over the chunk group `cs` (1 or 2 chunks).
            `subs[i]` is the post-op split width for chunk cs[i].
            `split_gate` runs the gate matmuls as two half-width
            sub-chains so the first half's a/z overlap the second
            half's matmuls (used for the very last chain only)."""
            xls = [xts_by_chunk[c] for c in cs]
            # ---- hidden channel-tiles (o = JT+j) FIRST
            phs = [ppool.tile([128, CH], F32, tag="psum", name=f"ph{i}")
                   for i in range(len(cs))]
            for k in range(KT):
                for i in range(len(cs)):
                    nc.tensor.matmul(
                        phs[i][:],
                        lhsT=wts[JT + j][:, k * 128:(k + 1) * 128],
                        rhs=xls[i][k](0, CH),
                        start=(k == 0),
                        stop=(k == KT - 1),
                    )
            gs = {}
            for i, c in enumerate(cs):
                for f0 in range(0, CH, subs[i]):
                    fs = slice(f0, f0 + subs[i])
                    sg_t = spool.tile([128, subs[i]], F32, tag="sg")
                    nc.scalar.activation(sg_t[:], phs[i][:, fs], AF.Sigmoid,
                                         bias=bias_t[:, JT + j:JT + j + 1],
                                         scale=1.0)
                    r_t = spool.tile([128, subs[i]], F32, tag="r")
                    nc.scalar.activation(r_t[:], phs[i][:, fs], AF.Relu,
                                         bias=bias_t[:, JT + j:JT + j + 1],
                                         scale=1.0)
                    # g = min(sigmoid(hidden), 0.5) + relu(hidden)
                    g_t = spool.tile([128, subs[i]], F32, tag="g")
                    nc.vector.scalar_tensor_tensor(g_t[:], sg_t[:], 0.5,
                                                   r_t[:], op0=OP.min,
                                                   op1=OP.add)
                    gs[(i, f0)] = g_t
            # ---- gate channel-tiles (overlap the hidden post-ops)
            pgs = [ppool.tile([128, CH], F32, tag="psum", name=f"pg{i}")
                   for i in range(len(cs))]
            gate_cols = ((0, CH),) if not split_gate else \
                ((0, CH // 2), (CH // 2, CH))
            for lo, hi in gate_cols:
                for k in range(KT):
                    for i in range(len(cs)):
                        nc.tensor.matmul(
                            pgs[i][:, lo:hi],
                            lhsT=wts[j][:, k * 128:(k + 1) * 128],
                            rhs=xls[i][k](lo, hi),
                            start=(k == 0),
                            stop=(k == KT - 1),
                        )
            for i, c in enumerate(cs):
                s0 = c * CH
                h_t = hpool.tile([128, CH], F32, tag=f"h{j}")
                for f0 in range(0, CH, subs[i]):
                    fs = slice(f0, f0 + subs[i])
                    a_t = abpool.tile([128, subs[i]], F32, tag="a")
                    nc.scalar.activation(a_t[:], pgs[i][:, fs], AF.Sigmoid,
                                         bias=nbias_t[:, j:j + 1], scale=-1.0)
                    z_t = spool.tile([128, subs[i]], F32, tag="z")
                    nc.scalar.activation(z_t[:], pgs[i][:, fs], AF.Sigmoid,
                                         bias=bias_t[:, j:j + 1], scale=1.0)
                    b_t = abpool.tile([128, subs[i]], F32, tag="b")
                    nc.vector.tensor_mul(b_t[:], z_t[:], gs[(i, f0)][:])
                    # ---- scan: h = a*h_prev + b along time
                    if f0 == 0:
                        init = (h0_t[:, j:j + 1] if c == 0
                                else prev_h[j][:, CH - 1:CH])
                    else:
                        init = h_t[:, f0 - 1:f0]
                    nc.vector.tensor_tensor_scan(h_t[:, fs], a_t[:], b_t[:],
                                                 init, op0=OP.mult, op1=OP.add)
                prev_h[j] = h_t
                # GpSimd's end-of-kernel DRAIN detects DMA completion
                # slowly (~6 us); keep the final chunks' stores on Sync
                # (idle by then) so the kernel end isn't gated on it.
                # The final chunk stores per-half so the last transfer
                # is short.
                out_q = nc.gpsimd if c < NCHUNK - 2 else nc.sync
                if c == NCHUNK - 1:
                    for f0 in range(0, CH, subs[i]):
                        out_q.dma_start(
                            out[j * 128:(j + 1) * 128,
                                s0 + f0:s0 + f0 + subs[i]],
                            h_t[:, f0:f0 + subs[i]])
                else:
                    out_q.dma_start(out[j * 128:(j + 1) * 128, s0:s0 + CH],
                                    h_t[:])

        # Chunks 0/1: single-chunk chains, interleaved so chains line
        # up with the W/x arrival schedule.
        for c, j in ([(0, j) for j in range(6)] + [(1, j) for j in range(4)]
                     + [(0, 6), (0, 7)] + [(1, j) for j in range(4, 8)]):
            chain((c,), j, (CH,))
        # Chunks 2..7 streamed singly with batched x transfers.  Only
        # the final two chains split their post-ops (halving EVERY last-
        # chunk chain's op width doubles ACT's fixed-overhead load and
        # makes ACT the tail bottleneck), and the very last chain also
        # splits its gate matmuls so a/z overlap them.
        for c in range(2, NCHUNK):
            x_load_big(c)
            last = c == NCHUNK - 1
            for j in range(JT):
                fine = last and j >= JT - 2
                chain((c,), j, (CH // 2 if fine else CH,),
                      split_gate=(last and j == JT - 1))
    nc.compile()
    return nc


def _run(inputs, trace=False, **spmd_kwargs):
    x = np.asarray(inputs["x"], dtype=np.float32)
    h = np.asarray(inputs["h"], dtype=np.float32)
    W = np.asarray(inputs["W"], dtype=np.float32)
    b = np.asarray(inputs["b"], dtype=np.float32)

    xt_all = np.ascontiguousarray(x.transpose(0, 2, 1)).reshape(
        B, KT, 128, S)                                             # (B,KT,128,S)
    # wp[o*128+p, k*128+c] = W[o*128+c, k*128+p]
    WP = np.ascontiguousarray(
        W.reshape(2 * JT, 128, KT, 128).transpose(0, 3, 2, 1)
        .reshape(2 * DH, DIN))
    bias_t = np.ascontiguousarray(b.reshape(2 * JT, 128).T)        # (128, 2JT)
    nbias_t = np.ascontiguousarray(-bias_t)
    h0_all = np.ascontiguousarray(
        h[:, 0, :].reshape(B, JT, 128).transpose(0, 2, 1))         # (B, 128, JT)

    if "prog" not in _prog_cache:
        _prog_cache["prog"] = _build_program()
    nc = _prog_cache["prog"]

    in_maps = [
        {"xt": xt_all[c], "wp": WP, "bias": bias_t, "nbias": nbias_t,
         "h0": h0_all[c]}
        for c in range(B)
    ]
    res = run_bass_kernel_spmd(nc, in_maps, list(range(B)), trace=trace,
                               **spmd_kwargs)
    out = np.stack([res.results[c]["out"].T for c in range(B)], axis=0)
    return np.ascontiguousarray(out), res


def kernel(**inputs) -> np.ndarray:
    return _run(inputs)[0]


# revision 29
# speedup vs baseline: 1.0158x; 1.0158x over previous
"""minGRU cell kernel for 8 Trainium2 NeuronCores.

Math (per batch b, all in linear domain — the recurrence is a convex
combination of positive values, so no log-space is needed):
    gh[s, :] = x[s, :] @ W.T + b          # (S, 2H)
    gate, hidden = gh[:, :H], gh[:, H:]
    z = sigmoid(gate);  a = 1 - z = sigmoid(-gate)
    g(hidden) = relu(hidden) + min(sigmoid(hidden), 0.5)
    h_t = a_t * h_{t-1} + z_t * g_t       # scan over s

Distribution: pure data parallel over B (8 batches -> 8 cores).

Device layout: channels on SBUF partitions, time on the free dim:
    out[o, s] = sum_i WT[i, o] * xT[i, s]
so the matmul result lands directly in the layout the DVE
tensor_tensor_scan instruction needs.  Matmuls run as float32r
(full-rate fp32 PE mode).

Perf structure (vs the naive version):
  * W is packed host-side per output-tile (o-major), so the first
    j-chain only needs 1 MB of W + 2.1 MB of x before the PE can
    start — the old k-major layout needed all 8.4 MB of W first
    (26 us of PE idle at startup).
  * DMA descriptor issue is spread across three queues (W on the
    Vector queue, x on Sync, consts + output stores on GpSimd) so
    descriptor serialization never gates the startup transfers.
  * Within each (chunk, j) iteration the hidden chain runs BEFORE the
    gate chain: the post-matmul serial tail (sigmoid/relu/min-add on
    the hidden projection) overlaps the gate matmuls, and the final
    chunk's post-ops are split into 256-column halves, shortening the
    end-of-kernel drain.
"""

from contextlib import ExitStack

import numpy as np

import concourse.bass as bass
import concourse.bacc as bacc
import concourse.mybir as mybir
import concourse.tile as tile
from concourse.bass_utils import run_bass_kernel_spmd

B, S, DIN, DH = 8, 4096, 1024, 1024
CH = 512                 # time-chunk (free dim of each matmul / scan)
NCHUNK = S // CH         # 8
KT = DIN // 128          # 8 contraction tiles
JT = DH // 128           # 8 channel tiles (per gate/hidden half)

F32 = mybir.dt.float32
F32R = mybir.dt.float32r
AF = mybir.ActivationFunctionType
OP = mybir.AluOpType

_prog_cache = {}


def _build_program() -> bass.Bass:
    nc = bacc.Bacc("TRN2", target_bir_lowering=False, debug=False,
                   num_devices=B)
    xt = nc.dram_tensor("xt", (KT, 128, S), F32R, kind="ExternalInput")
    # per-o packed weights: wp[o*128+p, k*128+c] = W[o*128+c, k*128+p]
    wp = nc.dram_tensor("wp", (2 * DH, DIN), F32R, kind="ExternalInput")
    bias = nc.dram_tensor("bias", (128, 2 * JT), F32, kind="ExternalInput")
    nbias = nc.dram_tensor("nbias", (128, 2 * JT), F32, kind="ExternalInput")
    h0 = nc.dram_tensor("h0", (128, JT), F32, kind="ExternalInput")
    out = nc.dram_tensor("out", (DH, S), F32, kind="ExternalOutput")

    with ExitStack() as ctx:
        tc = ctx.enter_context(tile.TileContext(nc))
        cpool = ctx.enter_context(tc.tile_pool(name="const", bufs=1))
        wpool = ctx.enter_context(tc.tile_pool(name="w", bufs=1))
        xpool = ctx.enter_context(tc.tile_pool(name="x", bufs=2))
        spool = ctx.enter_context(tc.tile_pool(name="tmp", bufs=2))
        abpool = ctx.enter_context(tc.tile_pool(name="ab", bufs=3))
        hpool = ctx.enter_context(tc.tile_pool(name="h", bufs=2))
        # 2 tiles per chain, bufs=4 => exactly 2 chains in flight.  More
        # (bufs=8) lets the PE run 4 chains ahead of ACT, and the
        # then-permanent concurrent ACT psum reads slow every matmul's
        # accumulation by ~20% (measured 227 -> 272 ns cadence).
        ppool = ctx.enter_context(tc.tile_pool(name="psum", bufs=4, space="PSUM"))

        # Consts go over the (otherwise idle) GpSimd queue, then are
        # re-materialized on the engines that consume them (ACT for
        # bias/nbias, DVE for h0) so hot-loop instructions never carry
        # a DMA sync-wait.
        bias_d = cpool.tile([128, 2 * JT], F32, tag="bias_d")
        nc.gpsimd.dma_start(bias_d[:], bias[:, :])
        nbias_d = cpool.tile([128, 2 * JT], F32, tag="nbias_d")
        nc.gpsimd.dma_start(nbias_d[:], nbias[:, :])
        h0_d = cpool.tile([128, JT], F32, tag="h0_d")
        nc.gpsimd.dma_start(h0_d[:], h0[:, :])

        # Startup is DMA-bandwidth-bound: ~12.6 MB (W + x chunks 0/1)
        # must land in the first ~40 us.  Startup-critical input
        # transfers are sequenced in exact need order so later
        # transfers never steal bandwidth from earlier ones:
        #   w(j=0), x chunk 0, w(j=1..5), x chunk 1, w(j=6,7)
        # The first batch is split across the Sync AND Scalar queues
        # (two descriptors in flight ramps the DMA engines up faster);
        # everything after runs on Sync alone so the Scalar queue is
        # free for the ACT hot loop by the time psum drains start.
        def w_load(o, queue=nc.sync, per_k=False):
            w_t = wpool.tile([128, DIN], F32R, tag=f"w{o}")
            if per_k:
                # per-k-slice descriptors: the first matmul only waits
                # for the first 64 KB, not the whole 512 KB tile.
                for k in range(KT):
                    queue.dma_start(w_t[:, k * 128:(k + 1) * 128],
                                    wp[o * 128:(o + 1) * 128,
                                       k * 128:(k + 1) * 128])
            else:
                queue.dma_start(w_t[:], wp[o * 128:(o + 1) * 128, :])
            wts[o] = w_t

        def x_load(c, alternate=False):
            # per-k descriptors: chunk-0 chains consume tiles
            # k-progressively as they land.
            s0 = c * CH
            xts = []
            for k in range(KT):
                x_t = xpool.tile([128, CH], F32R, tag=f"x{k}")
                q = nc.scalar if (alternate and k % 2 == 1) else nc.sync
                q.dma_start(x_t[:], xt[k, :, s0:s0 + CH])
                xts.append(x_t)
            xts_by_chunk[c] = [
                (lambda t: (lambda lo, hi: t[:, lo:hi]))(x_t) for x_t in xts]

        def x_load_big(c):
            # steady-state chunks: ONE descriptor for the whole chunk
            # (8 strided k-blocks) => one DMA semaphore instead of 8 on
            # the Tensor queue.
            s0 = c * CH
            xbig = xpool.tile([128, KT * CH], F32R, tag="xbig")
            nc.sync.dma_start(xbig[:].rearrange("p (k t) -> p k t", k=KT),
                              xt[:, :, s0:s0 + CH].rearrange("k p t -> p k t"))
            xts_by_chunk[c] = [
                (lambda kk: (lambda lo, hi:
                             xbig[:, kk * CH + lo:kk * CH + hi]))(k)
                for k in range(KT)]

        wts = [None] * (2 * JT)
        xts_by_chunk = {}
        # Chunk-0's end is data-bound (w pair + 2.1 MB of x) no matter
        # how early the first matmul fires, so keep the simple smooth
        # split: w tiles whole on Sync, x chunk 0 alternating between
        # Sync and Scalar, gate w(j=0) on Scalar.
        w_load(JT + 0)
        w_load(0, nc.scalar)
        x_load(0, alternate=True)
        for j in range(1, 6):
            w_load(JT + j), w_load(j)
        x_load(1)
        for j in range(6, 8):
            w_load(JT + j), w_load(j)

        # Const copies: first ACT op needs bias_t at ~17 us.
        bias_t = cpool.tile([128, 2 * JT], F32, tag="bias")
        nc.scalar.copy(bias_t[:], bias_d[:])
        nbias_t = cpool.tile([128, 2 * JT], F32, tag="nbias")
        nc.scalar.copy(nbias_t[:], nbias_d[:])
        h0_t = cpool.tile([128, JT], F32, tag="h0")
        nc.vector.tensor_copy(h0_t[:], h0_d[:])

        prev_h = [None] * JT

        def chain(cs, j, subs, split_gate=False):
            """One j-chain over the chunk group `cs` (1 or 2 chunks).
            `subs[i]` is the post-op split width for chunk cs[i].
            `split_gate` runs the gate matmuls as two half-width
            sub-chains so the first half's a/z overlap the second
            half's matmuls (used for the very last chain only)."""
            xls = [xts_by_chunk[c] for c in cs]
            # ---- hidden channel-tiles (o = JT+j) FIRST
            phs = [ppool.tile([128, CH], F32, tag="psum", name=f"ph{i}")
                   for i in range(len(cs))]
            for k in range(KT):
                for i in range(len(cs)):
                    nc.tensor.matmul(
                        phs[i][:],
                        lhsT=wts[JT + j][:, k * 128:(k + 1) * 128],
                        rhs=xls[i][k](0, CH),
                        start=(k == 0),
                        stop=(k == KT - 1),
                    )
            gs = {}
            for i, c in enumerate(cs):
                for f0 in range(0, CH, subs[i]):
                    fs = slice(f0, f0 + subs[i])
                    sg_t = spool.tile([128, subs[i]], F32, tag="sg")
                    nc.scalar.activation(sg_t[:], phs[i][:, fs], AF.Sigmoid,
                                         bias=bias_t[:, JT + j:JT + j + 1],
                                         scale=1.0)
                    r_t = spool.tile([128, subs[i]], F32, tag="r")
                    nc.scalar.activation(r_t[:], phs[i][:, fs], AF.Relu,
                                         bias=bias_t[:, JT + j:JT + j + 1],
                                         scale=1.0)
                    # g = min(sigmoid(hidden), 0.5) + relu(hidden)
                    g_t = spool.tile([128, subs[i]], F32, tag="g")
                    nc.vector.scalar_tensor_tensor(g_t[:], sg_t[:], 0.5,
                                                   r_t[:], op0=OP.min,
                                                   op1=OP.add)
                    gs[(i, f0)] = g_t
            # ---- gate channel-tiles (overlap the hidden post-ops)
            pgs = [ppool.tile([128, CH], F32, tag="psum", name=f"pg{i}")
                   for i in range(len(cs))]
            gate_cols = ((0, CH),) if not split_gate else \
                ((0, CH // 2), (CH // 2, CH))
            for lo, hi in gate_cols:
                for k in range(KT):
                    for i in range(len(cs)):
                        nc.tensor.matmul(
                            pgs[i][:, lo:hi],
                            lhsT=wts[j][:, k * 128:(k + 1) * 128],
                            rhs=xls[i][k](lo, hi),
                            start=(k == 0),
                            stop=(k == KT - 1),
                        )
            for i, c in enumerate(cs):
                s0 = c * CH
                h_t = hpool.tile([128, CH], F32, tag=f"h{j}")
                for f0 in range(0, CH, subs[i]):
                    fs = slice(f0, f0 + subs[i])
                    a_t = abpool.tile([128, subs[i]], F32, tag="a")
                    nc.scalar.activation(a_t[:], pgs[i][:, fs], AF.Sigmoid,
                                         bias=nbias_t[:, j:j + 1], scale=-1.0)
                    z_t = spool.tile([128, subs[i]], F32, tag="z")
                    nc.scalar.activation(z_t[:], pgs[i][:, fs], AF.Sigmoid,
                                         bias=bias_t[:, j:j + 1], scale=1.0)
                    b_t = abpool.tile([128, subs[i]], F32, tag="b")
                    nc.vector.tensor_mul(b_t[:], z_t[:], gs[(i, f0)][:])
                    # ---- scan: h = a*h_prev + b along time
                    if f0 == 0:
                        init = (h0_t[:, j:j + 1] if c == 0
                                else prev_h[j][:, CH - 1:CH])
                    else:
                        init = h_t[:, f0 - 1:f0]
                    nc.vector.tensor_tensor_scan(h_t[:, fs], a_t[:], b_t[:],
                                                 init, op0=OP.mult, op1=OP.add)
                prev_h[j] = h_t
                # GpSimd's end-of-kernel DRAIN detects DMA completion
                # slowly (~6 us); keep the final chunks' stores on Sync
                # (idle by then) so the kernel end isn't gated on it.
                # The final chunk stores per-half so the last transfer
                # is short.
                out_q = nc.gpsimd if c < NCHUNK - 2 else nc.scalar
                if c == NCHUNK - 1:
                    for f0 in range(0, CH, subs[i]):
                        out_q.dma_start(
                            out[j * 128:(j + 1) * 128,
                                s0 + f0:s0 + f0 + subs[i]],
                            h_t[:, f0:f0 + subs[i]])
                else:
                    out_q.dma_start(out[j * 128:(j + 1) * 128, s0:s0 + CH],
                                    h_t[:])

        # Chunks 0/1: single-chunk chains, interleaved so chains line
        # up with the W/x arrival schedule.
        for c, j in ([(0, j) for j in range(6)] + [(1, j) for j in range(4)]
                     + [(0, 6), (0, 7)] + [(1, j) for j in range(4, 8)]):
            chain((c,), j, (CH,))
        # Chunks 2..7 streamed singly with batched x transfers.  Only
        # the final two chains split their post-ops (halving EVERY last-
        # chunk chain's op width doubles ACT's fixed-overhead load and
        # makes ACT the tail bottleneck), and the very last chain also
        # splits its gate matmuls so a/z overlap them.
        for c in range(2, NCHUNK):
            x_load_big(c)
            last = c == NCHUNK - 1
            for j in range(JT):
                fine = last and j >= JT - 2
                chain((c,), j, (CH // 2 if fine else CH,),
                      split_gate=(last and j == JT - 1))
    nc.compile()
    return nc


def _run(inputs, trace=False, **spmd_kwargs):
    x = np.asarray(inputs["x"], dtype=np.float32)
    h = np.asarray(inputs["h"], dtype=np.float32)
    W = np.asarray(inputs["W"], dtype=np.float32)
    b = np.asarray(inputs["b"], dtype=np.float32)

    xt_all = np.ascontiguousarray(x.transpose(0, 2, 1)).reshape(
        B, KT, 128, S)                                             # (B,KT,128,S)
    # wp[o*128+p, k*128+c] = W[o*128+c, k*128+p]
    WP = np.ascontiguousarray(
        W.reshape(2 * JT, 128, KT, 128).transpose(0, 3, 2, 1)
        .reshape(2 * DH, DIN))
    bias_t = np.ascontiguousarray(b.reshape(2 * JT, 128).T)        # (128, 2JT)
    nbias_t = np.ascontiguousarray(-bias_t)
    h0_all = np.ascontiguousarray(
        h[:, 0, :].reshape(B, JT, 128).transpose(0, 2, 1))         # (B, 128, JT)

    if "prog" not in _prog_cache:
        _prog_cache["prog"] = _build_program()
    nc = _prog_cache["prog"]

    in_maps = [
        {"xt": xt_all[c], "wp": WP, "bias": bias_t, "nbias": nbias_t,
         "h0": h0_all[c]}
        for c in range(B)
    ]
    res = run_bass_kernel_spmd(nc, in_maps, list(range(B)), trace=trace,
                               **spmd_kwargs)
    out = np.stack([res.results[c]["out"].T for c in range(B)], axis=0)
    return np.ascontiguousarray(out), res


def kernel(**inputs) -> np.ndarray:
    return _run(inputs)[0]


# revision 30
# speedup vs baseline: 1.0221x; 1.0062x over previous
"""minGRU cell kernel for 8 Trainium2 NeuronCores.

Math (per batch b, all in linear domain — the recurrence is a convex
combination of positive values, so no log-space is needed):
    gh[s, :] = x[s, :] @ W.T + b          # (S, 2H)
    gate, hidden = gh[:, :H], gh[:, H:]
    z = sigmoid(gate);  a = 1 - z = sigmoid(-gate)
    g(hidden) = relu(hidden) + min(sigmoid(hidden), 0.5)
    h_t = a_t * h_{t-1} + z_t * g_t       # scan over s

Distribution: pure data parallel over B (8 batches -> 8 cores).

Device layout: channels on SBUF partitions, time on the free dim:
    out[o, s] = sum_i WT[i, o] * xT[i, s]
so the matmul result lands directly in the layout the DVE
tensor_tensor_scan instruction needs.  Matmuls run as float32r
(full-rate fp32 PE mode).

Perf structure (vs the naive version):
  * W is packed host-side per output-tile (o-major), so the first
    j-chain only needs 1 MB of W + 2.1 MB of x before the PE can
    start — the old k-major layout needed all 8.4 MB of W first
    (26 us of PE idle at startup).
  * DMA descriptor issue is spread across three queues (W on the
    Vector queue, x on Sync, consts + output stores on GpSimd) so
    descriptor serialization never gates the startup transfers.
  * Within each (chunk, j) iteration the hidden chain runs BEFORE the
    gate chain: the post-matmul serial tail (sigmoid/relu/min-add on
    the hidden projection) overlaps the gate matmuls, and the final
    chunk's post-ops are split into 256-column halves, shortening the
    end-of-kernel drain.
"""

from contextlib import ExitStack

import numpy as np

import concourse.bass as bass
import concourse.bacc as bacc
import concourse.mybir as mybir
import concourse.tile as tile
from concourse.bass_utils import run_bass_kernel_spmd

B, S, DIN, DH = 8, 4096, 1024, 1024
CH = 512                 # time-chunk (free dim of each matmul / scan)
NCHUNK = S // CH         # 8
KT = DIN // 128          # 8 contraction tiles
JT = DH // 128           # 8 channel tiles (per gate/hidden half)

F32 = mybir.dt.float32
F32R = mybir.dt.float32r
AF = mybir.ActivationFunctionType
OP = mybir.AluOpType

_prog_cache = {}


def _build_program() -> bass.Bass:
    nc = bacc.Bacc("TRN2", target_bir_lowering=False, debug=False,
                   num_devices=B)
    xt = nc.dram_tensor("xt", (KT, 128, S), F32R, kind="ExternalInput")
    # per-o packed weights: wp[o*128+p, k*128+c] = W[o*128+c, k*128+p]
    wp = nc.dram_tensor("wp", (2 * DH, DIN), F32R, kind="ExternalInput")
    bias = nc.dram_tensor("bias", (128, 2 * JT), F32, kind="ExternalInput")
    nbias = nc.dram_tensor("nbias", (128, 2 * JT), F32, kind="ExternalInput")
    h0 = nc.dram_tensor("h0", (128, JT), F32, kind="ExternalInput")
    out = nc.dram_tensor("out", (DH, S), F32, kind="ExternalOutput")

    with ExitStack() as ctx:
        tc = ctx.enter_context(tile.TileContext(nc))
        cpool = ctx.enter_context(tc.tile_pool(name="const", bufs=1))
        wpool = ctx.enter_context(tc.tile_pool(name="w", bufs=1))
        xpool = ctx.enter_context(tc.tile_pool(name="x", bufs=2))
        spool = ctx.enter_context(tc.tile_pool(name="tmp", bufs=2))
        abpool = ctx.enter_context(tc.tile_pool(name="ab", bufs=3))
        hpool = ctx.enter_context(tc.tile_pool(name="h", bufs=2))
        # 2 tiles per chain, bufs=4 => exactly 2 chains in flight.  More
        # (bufs=8) lets the PE run 4 chains ahead of ACT, and the
        # then-permanent concurrent ACT psum reads slow every matmul's
        # accumulation by ~20% (measured 227 -> 272 ns cadence).
        ppool = ctx.enter_context(tc.tile_pool(name="psum", bufs=4, space="PSUM"))

        # Consts go over the (otherwise idle) GpSimd queue, then are
        # re-materialized on the engines that consume them (ACT for
        # bias/nbias, DVE for h0) so hot-loop instructions never carry
        # a DMA sync-wait.
        bias_d = cpool.tile([128, 2 * JT], F32, tag="bias_d")
        nc.gpsimd.dma_start(bias_d[:], bias[:, :])
        nbias_d = cpool.tile([128, 2 * JT], F32, tag="nbias_d")
        nc.gpsimd.dma_start(nbias_d[:], nbias[:, :])
        h0_d = cpool.tile([128, JT], F32, tag="h0_d")
        nc.gpsimd.dma_start(h0_d[:], h0[:, :])

        # Startup is DMA-bandwidth-bound: ~12.6 MB (W + x chunks 0/1)
        # must land in the first ~40 us.  Startup-critical input
        # transfers are sequenced in exact need order so later
        # transfers never steal bandwidth from earlier ones:
        #   w(j=0), x chunk 0, w(j=1..5), x chunk 1, w(j=6,7)
        # The first batch is split across the Sync AND Scalar queues
        # (two descriptors in flight ramps the DMA engines up faster);
        # everything after runs on Sync alone so the Scalar queue is
        # free for the ACT hot loop by the time psum drains start.
        def w_load(o, queue=nc.sync, per_k=False):
            w_t = wpool.tile([128, DIN], F32R, tag=f"w{o}")
            if per_k:
                # per-k-slice descriptors: the first matmul only waits
                # for the first 64 KB, not the whole 512 KB tile.
                for k in range(KT):
                    queue.dma_start(w_t[:, k * 128:(k + 1) * 128],
                                    wp[o * 128:(o + 1) * 128,
                                       k * 128:(k + 1) * 128])
            else:
                queue.dma_start(w_t[:], wp[o * 128:(o + 1) * 128, :])
            wts[o] = w_t

        def x_load(c, alternate=False):
            # per-k descriptors: chunk-0 chains consume tiles
            # k-progressively as they land.
            s0 = c * CH
            xts = []
            for k in range(KT):
                x_t = xpool.tile([128, CH], F32R, tag=f"x{k}")
                q = nc.scalar if (alternate and k % 2 == 1) else nc.sync
                q.dma_start(x_t[:], xt[k, :, s0:s0 + CH])
                xts.append(x_t)
            xts_by_chunk[c] = [
                (lambda t: (lambda lo, hi: t[:, lo:hi]))(x_t) for x_t in xts]

        def x_load_big(c):
            # steady-state chunks: ONE descriptor for the whole chunk
            # (8 strided k-blocks) => one DMA semaphore instead of 8 on
            # the Tensor queue.
            s0 = c * CH
            xbig = xpool.tile([128, KT * CH], F32R, tag="xbig")
            nc.sync.dma_start(xbig[:].rearrange("p (k t) -> p k t", k=KT),
                              xt[:, :, s0:s0 + CH].rearrange("k p t -> p k t"))
            xts_by_chunk[c] = [
                (lambda kk: (lambda lo, hi:
                             xbig[:, kk * CH + lo:kk * CH + hi]))(k)
                for k in range(KT)]

        wts = [None] * (2 * JT)
        xts_by_chunk = {}
        # Chunk-0's end is data-bound (w pair + 2.1 MB of x) no matter
        # how early the first matmul fires, so keep the simple smooth
        # split: w tiles whole on Sync, x chunk 0 alternating between
        # Sync and Scalar, gate w(j=0) on Scalar.
        w_load(JT + 0)
        w_load(0, nc.scalar)
        x_load(0, alternate=True)
        for j in range(1, 6):
            w_load(JT + j), w_load(j)
        x_load(1)
        for j in range(6, 8):
            w_load(JT + j), w_load(j)

        # Const copies: first ACT op needs bias_t at ~17 us.
        bias_t = cpool.tile([128, 2 * JT], F32, tag="bias")
        nc.scalar.copy(bias_t[:], bias_d[:])
        nbias_t = cpool.tile([128, 2 * JT], F32, tag="nbias")
        nc.scalar.copy(nbias_t[:], nbias_d[:])
        h0_t = cpool.tile([128, JT], F32, tag="h0")
        nc.vector.tensor_copy(h0_t[:], h0_d[:])

        prev_h = [None] * JT

        def chain(cs, j, subs, split_gate=False):
            """One j-chain over the chunk group `cs` (1 or 2 chunks).
            `subs[i]` is the post-op split width for chunk cs[i].
            `split_gate` runs the gate matmuls as two half-width
            sub-chains so the first half's a/z overlap the second
            half's matmuls (used for the very last chain only)."""
            xls = [xts_by_chunk[c] for c in cs]
            # ---- hidden channel-tiles (o = JT+j) FIRST
            phs = [ppool.tile([128, CH], F32, tag="psum", name=f"ph{i}")
                   for i in range(len(cs))]
            for k in range(KT):
                for i in range(len(cs)):
                    nc.tensor.matmul(
                        phs[i][:],
                        lhsT=wts[JT + j][:, k * 128:(k + 1) * 128],
                        rhs=xls[i][k](0, CH),
                        start=(k == 0),
                        stop=(k == KT - 1),
                    )
            gs = {}
            for i, c in enumerate(cs):
                for f0 in range(0, CH, subs[i]):
                    fs = slice(f0, f0 + subs[i])
                    sg_t = spool.tile([128, subs[i]], F32, tag="sg")
                    nc.scalar.activation(sg_t[:], phs[i][:, fs], AF.Sigmoid,
                                         bias=bias_t[:, JT + j:JT + j + 1],
                                         scale=1.0)
                    r_t = spool.tile([128, subs[i]], F32, tag="r")
                    nc.scalar.activation(r_t[:], phs[i][:, fs], AF.Relu,
                                         bias=bias_t[:, JT + j:JT + j + 1],
                                         scale=1.0)
                    # g = min(sigmoid(hidden), 0.5) + relu(hidden)
                    g_t = spool.tile([128, subs[i]], F32, tag="g")
                    nc.vector.scalar_tensor_tensor(g_t[:], sg_t[:], 0.5,
                                                   r_t[:], op0=OP.min,
                                                   op1=OP.add)
                    gs[(i, f0)] = g_t
            # ---- gate channel-tiles (overlap the hidden post-ops)
            pgs = [ppool.tile([128, CH], F32, tag="psum", name=f"pg{i}")
                   for i in range(len(cs))]
            gate_cols = ((0, CH),) if not split_gate else \
                ((0, CH // 2), (CH // 2, CH))
            for lo, hi in gate_cols:
                for k in range(KT):
                    for i in range(len(cs)):
                        nc.tensor.matmul(
                            pgs[i][:, lo:hi],
                            lhsT=wts[j][:, k * 128:(k + 1) * 128],
                            rhs=xls[i][k](lo, hi),
                            start=(k == 0),
                            stop=(k == KT - 1),
                        )
            for i, c in enumerate(cs):
                s0 = c * CH
                h_t = hpool.tile([128, CH], F32, tag=f"h{j}")
                for f0 in range(0, CH, subs[i]):
                    fs = slice(f0, f0 + subs[i])
                    a_t = abpool.tile([128, subs[i]], F32, tag="a")
                    nc.scalar.activation(a_t[:], pgs[i][:, fs], AF.Sigmoid,
                                         bias=nbias_t[:, j:j + 1], scale=-1.0)
                    z_t = spool.tile([128, subs[i]], F32, tag="z")
                    nc.scalar.activation(z_t[:], pgs[i][:, fs], AF.Sigmoid,
                                         bias=bias_t[:, j:j + 1], scale=1.0)
                    b_t = abpool.tile([128, subs[i]], F32, tag="b")
                    nc.vector.tensor_mul(b_t[:], z_t[:], gs[(i, f0)][:])
                    # ---- scan: h = a*h_prev + b along time
                    if f0 == 0:
                        init = (h0_t[:, j:j + 1] if c == 0
                                else prev_h[j][:, CH - 1:CH])
                    else:
                        init = h_t[:, f0 - 1:f0]
                    nc.vector.tensor_tensor_scan(h_t[:, fs], a_t[:], b_t[:],
                                                 init, op0=OP.mult, op1=OP.add)
                prev_h[j] = h_t
                # GpSimd's end-of-kernel DRAIN detects DMA completion
                # slowly (~6 us); keep the final chunks' stores on Sync
                # (idle by then) so the kernel end isn't gated on it.
                # The final chunk stores per-half so the last transfer
                # is short.
                out_q = nc.gpsimd if c < NCHUNK - 2 else nc.sync
                if c == NCHUNK - 1:
                    for f0 in range(0, CH, subs[i]):
                        out_q.dma_start(
                            out[j * 128:(j + 1) * 128,
                                s0 + f0:s0 + f0 + subs[i]],
                            h_t[:, f0:f0 + subs[i]])
                else:
                    out_q.dma_start(out[j * 128:(j + 1) * 128, s0:s0 + CH],
                                    h_t[:])

        # Chunks 0/1: single-chunk chains, interleaved so chains line
        # up with the W/x arrival schedule.
        for c, j in ([(0, j) for j in range(6)] + [(1, j) for j in range(4)]
                     + [(0, 6), (0, 7)] + [(1, j) for j in range(4, 8)]):
            chain((c,), j, (CH,))
        # Chunks 2..7 streamed singly with batched x transfers.  Only
        # the final two chains split their post-ops (halving EVERY last-
        # chunk chain's op width doubles ACT's fixed-overhead load and
        # makes ACT the tail bottleneck), and the very last chain also
        # splits its gate matmuls so a/z overlap them.
        for c in range(2, NCHUNK):
            x_load_big(c)
            last = c == NCHUNK - 1
            for j in range(JT):
                fine = last and j >= JT - 2
                chain((c,), j, (CH // 2 if fine else CH,),
                      split_gate=(last and j == JT - 1))
    nc.compile()
    return nc


def _run(inputs, trace=False, **spmd_kwargs):
    x = np.asarray(inputs["x"], dtype=np.float32)
    h = np.asarray(inputs["h"], dtype=np.float32)
    W = np.asarray(inputs["W"], dtype=np.float32)
    b = np.asarray(inputs["b"], dtype=np.float32)

    xt_all = np.ascontiguousarray(x.transpose(0, 2, 1)).reshape(
        B, KT, 128, S)                                             # (B,KT,128,S)
    # wp[o*128+p, k*128+c] = W[o*128+c, k*128+p]
    WP = np.ascontiguousarray(
        W.reshape(2 * JT, 128, KT, 128).transpose(0, 3, 2, 1)
        .reshape(2 * DH, DIN))
    bias_t = np.ascontiguousarray(b.reshape(2 * JT, 128).T)        # (128, 2JT)
    nbias_t = np.ascontiguousarray(-bias_t)
    h0_all = np.ascontiguousarray(
        h[:, 0, :].reshape(B, JT, 128).transpose(0, 2, 1))         # (B, 128, JT)

    if "prog" not in _prog_cache:
        _prog_cache["prog"] = _build_program()
    nc = _prog_cache["prog"]

    in_maps = [
        {"xt": xt_all[c], "wp": WP, "bias": bias_t, "nbias": nbias_t,
         "h0": h0_all[c]}
        for c in range(B)
    ]
    res = run_bass_kernel_spmd(nc, in_maps, list(range(B)), trace=trace,
                               **spmd_kwargs)
    out = np.stack([res.results[c]["out"].T for c in range(B)], axis=0)
    return np.ascontiguousarray(out), res


def kernel(**inputs) -> np.ndarray:
    return _run(inputs)[0]
